# revision 11
# baseline (speedup 1.0000x reference)
"""Trainium2 Bass kernel for a 2-layer dense GCN block:

    z = x.reshape(B, N, F)                     # B=4, N=8192, F=64
    for i in range(2):
        z = relu((A @ z) @ W_i)                # A: [N, N] dense
    return z

Strategy (8 NeuronCores, SPMD):
  * Shard the output rows (m) of A @ Z across cores: core j owns rows
    [1024*j, 1024*(j+1)).  The host hands core j the matching
    column-slice of A^T (contraction dim n on SBUF partitions), cast to
    bf16 and pre-swizzled into exact SBUF tile order so every chunk DMA
    is one flat contiguous copy (short strided runs halve HBM rate).
    The 16 MB shard stays resident in SBUF for BOTH layers -- A is read
    from HBM exactly once.
  * Ring assignment is built around the inter-layer AllGather chain,
    whose ~30-40 us ncfw latency per collective is the critical path:
    z0 + z1_loc stores + outputs ride the sync HWDGE ring (so a
    gather's input store -- which gates its trigger doorbell -- never
    queues behind the big A load), while A + gather reloads ride the
    scalar ring (a reload waiting on its collective has nothing queued
    behind it there).
  * Z is a [n, c] matrix with c = b*F + f (256 columns).  Layer matmuls
    compute H^T[c, m] = sum_n Z[n, c] * A^T[n, m] on the tensor engine
    (lhsT = Z tile stationary, rhs = A^T tile moving, fp32 PSUM accum).
  * Layer 1 runs as FOUR m-quarter passes; quarter q's AllGather
    triggers ~1/4 of the way through the layer and the 4-slice chain
    pipelines under the rest of layer 1 and the start of layer 2.
    Both c-halves of a pass accumulate into ONE psum bank (per-element
    has_written handles the shared bank with a single start=True), so
    layers use disjoint psum tags and a later pass's tail can never be
    scheduler-ordered behind the next layer's matmuls.
  * Layer 2 accumulation matmuls are emitted at LOW scheduler priority
    so layer-1 tail work (weight-apply -> relu -> store -> gather
    trigger) always precedes them in the PE queue -- a reload-waiting
    LDWEIGHTS ahead of them would wedge the whole PE FIFO.  Layer 2
    consumes gather slices in arrival order, in two m-half passes.
  * bf16 operands / fp32 accumulation throughout (~0.5% rel-l2 vs the
    fp32 reference).
"""

import contextlib

import numpy as np
import ml_dtypes

import concourse.mybir as mybir
import concourse.tile as tile
from concourse import bacc
from concourse.bass_utils import run_bass_kernel_spmd

BF16 = ml_dtypes.bfloat16

NCORES = 8
B, N, F, L = 4, 8192, 64, 2
C = B * F                      # 256 columns of the Z matrix
M_CORE = N // NCORES           # 1024 output rows per core
NT = N // 128                  # 64 contraction tiles of 128
MT = M_CORE // 128             # 8 output-row tiles of 128 per core
KCH = 4                        # DMA chunks for the resident A^T shard
TPC = NT // KCH                # 16 n-tiles per A chunk
ZCH = 16                       # DMA chunks for z0 (small, for fast start)
TPZ = NT // ZCH                # 4 n-tiles per z chunk
NQ = 4                         # m-quarter passes / gather slices
MPG = MT // NQ                 # m-tiles per gather slice (2)
MQ = M_CORE // NQ              # m columns per quarter (256)

_CACHED = {}


def _build_program():
    nc = bacc.Bacc("TRN2", target_bir_lowering=False, debug=False,
                   num_devices=NCORES)
    dt = mybir.dt

    at_d = nc.dram_tensor("at", [NQ * KCH * 128, TPC * MQ], dt.bfloat16,
                          kind="ExternalInput")
    z0_d = nc.dram_tensor("z0", [ZCH * 128, TPZ * C], dt.bfloat16,
                          kind="ExternalInput")
    w_d = nc.dram_tensor("w", [128, 2 * 128], dt.bfloat16, kind="ExternalInput")
    out_d = nc.dram_tensor("out", [M_CORE, C], dt.bfloat16, kind="ExternalOutput")

    z1_loc = nc.dram_tensor("z1_loc", [M_CORE, C], dt.bfloat16)
    warm_in = nc.dram_tensor("warm_in", [1, 128], dt.bfloat16)
    warm_out = nc.dram_tensor("warm_out", [NCORES, 128], dt.bfloat16)
    z1g = [nc.dram_tensor(f"z1g{g}", [NCORES * MPG * 128, C], dt.bfloat16)
           for g in range(NQ)]

    with tile.TileContext(nc) as tc:
        with tc.tile_pool(name="a_res", bufs=1) as a_pool, \
             tc.tile_pool(name="z_res", bufs=1) as z_pool, \
             tc.tile_pool(name="z1_res", bufs=1) as z1_pool, \
             tc.tile_pool(name="wk", bufs=1) as w_pool, \
             tc.tile_pool(name="ps", bufs=1, space="PSUM") as ps_pool, \
             tc.tile_pool(name="pz", bufs=2, space="PSUM") as psz_pool, \
             tc.tile_pool(name="hsb", bufs=2) as hsb_pool, \
             tc.tile_pool(name="zout", bufs=4) as zout_pool:

            w_sb = w_pool.tile([128, 2 * 128], dt.bfloat16, tag="w")
            nc.scalar.dma_start(out=w_sb[:], in_=w_d[:])

            at_sb = [[a_pool.tile([128, TPC * MQ], dt.bfloat16,
                                  tag=f"at{q}_{k}", name=f"at_sb{q}_{k}")
                      for k in range(KCH)] for q in range(NQ)]
            z_sb = [z_pool.tile([128, TPZ * C], dt.bfloat16,
                                tag=f"z{k}", name=f"z_sb{k}")
                    for k in range(ZCH)]
            z1_sb = [z1_pool.tile([128, NCORES * MPG * C], dt.bfloat16,
                                  tag=f"z1g{g}", name=f"z1_sb{g}")
                     for g in range(NQ)]

            for k in range(ZCH):
                nc.sync.dma_start(out=z_sb[k][:],
                                  in_=z0_d[k * 128:(k + 1) * 128, :])
            for q in range(NQ):
                for k in range(KCH):
                    r = (q * KCH + k) * 128
                    nc.scalar.dma_start(out=at_sb[q][k][:],
                                        in_=at_d[r:r + 128, :])

            def z_tile(t, ch):
                """lhsT: Z[n-tile t, c-half ch] -> [128, 128] bf16."""
                k, tt = divmod(t, TPZ)
                return z_sb[k][:, tt * C + ch * 128: tt * C + ch * 128 + 128]

            def z2_tile(t, ch):
                """Same, from the gathered z1 slices."""
                cb, r = divmod(t, MT)
                g, tt = divmod(r, MPG)
                blk = cb * MPG + tt
                return z1_sb[g][:, blk * C + ch * 128: blk * C + ch * 128 + 128]

            def at_tile(t, q):
                """rhs: A^T[n-tile t, m-quarter q] -> [128, 256] bf16."""
                k, tt = divmod(t, TPC)
                return at_sb[q][k][:, tt * MQ:(tt + 1) * MQ]

            h_sb = [hsb_pool.tile([128, M_CORE], dt.bfloat16,
                                  tag=f"h{ch}", name=f"h_sb{ch}")
                    for ch in range(2)]

            def tail(li, qs, h_ap, on_slice_done, prio):
                # weight apply + relu + store for the gather slices of a
                # just-finished pass; overlaps the next pass's matmuls.
                with prio:
                    for ch in range(2):
                        for q in qs:
                            nc.vector.tensor_copy(
                                h_sb[ch][:, q * MQ:(q + 1) * MQ],
                                h_ap(ch, q),
                            )
                    for g in qs:
                        z_ps = psz_pool.tile([128, MPG * C], dt.float32,
                                             tag="zps", name=f"z_ps_{li}_{g}")
                        for j in range(MPG):
                            i = g * MPG + j
                            for ch in range(2):
                                nc.tensor.matmul(
                                    z_ps[:, j * C + ch * 128:
                                         j * C + (ch + 1) * 128],
                                    h_sb[ch][:, i * 128:(i + 1) * 128],
                                    w_sb[:, li * 128:(li + 1) * 128],
                                    start=(j == 0 and ch == 0), stop=True,
                                )
                        z_o = zout_pool.tile([128, MPG * C], dt.bfloat16,
                                             tag="zo", name=f"z_o_{li}_{g}")
                        nc.scalar.activation(z_o[:], z_ps[:],
                                             mybir.ActivationFunctionType.Relu)
                        on_slice_done(g, z_o)

            # Warm the ncfw collective path early (hidden under the A
            # load).  Keep it TINY: a full-size warmup crawls against
            # the loaded HBM and the real gathers queue behind it.
            nc.gpsimd.dma_start(out=warm_in[:], in_=z0_d[0:1, 0:128])
            nc.gpsimd.collective_compute(
                "AllGather",
                mybir.AluOpType.bypass,
                replica_groups=[list(range(NCORES))],
                ins=[warm_in.ap().opt()],
                outs=[warm_out.ap().opt()],
            )

            # ---- layer 1: four m-quarter passes ----
            def l1_slice_done(g, z_o):
                nc.sync.dma_start(
                    out=z1_loc.ap()[g * MPG * 128:(g + 1) * MPG * 128, :]
                        .rearrange("(t p) c -> p t c", p=128),
                    in_=z_o.rearrange("p (t c) -> p t c", c=C))
                nc.gpsimd.collective_compute(
                    "AllGather",
                    mybir.AluOpType.bypass,
                    replica_groups=[list(range(NCORES))],
                    ins=[z1_loc.ap()[g * MPG * 128:(g + 1) * MPG * 128, :].opt()],
                    outs=[z1g[g].ap().opt()],
                )
                nc.scalar.dma_start(
                    out=z1_sb[g].rearrange("p (cb t c) -> p cb t c",
                                           cb=NCORES, t=MPG),
                    in_=z1g[g].ap().rearrange("(cb t p) c -> p cb t c",
                                              cb=NCORES, p=128))

            l1_ps = [ps_pool.tile([128, 2 * MQ], dt.float32, tag=f"hl1_{par}",
                                  name=f"l1_ps{par}") for par in range(2)]
            for q in range(NQ):
                par = q % 2
                for ti, t in enumerate(range(NT)):
                    for ch in range(2):
                        nc.tensor.matmul(
                            l1_ps[par][:, ch * MQ:(ch + 1) * MQ],
                            z_tile(t, ch),
                            at_tile(t, q),
                            start=(ti == 0 and ch == 0),
                            stop=(ti == NT - 1),
                        )
                tail(0, [q],
                     lambda ch, _q, par=par: l1_ps[par][:, ch * MQ:(ch + 1) * MQ],
                     l1_slice_done, tc.high_priority())

            # ---- layer 2: two m-half passes, n-tiles in gather order ----
            t2 = [MT * cb + MPG * g + tt
                  for g in range(NQ) for cb in range(NCORES)
                  for tt in range(MPG)]

            def l2_slice_done(g, z_o):
                nc.sync.dma_start(
                    out=out_d.ap()[g * MPG * 128:(g + 1) * MPG * 128, :]
                        .rearrange("(t p) c -> p t c", p=128),
                    in_=z_o.rearrange("p (t c) -> p t c", c=C))

            for mh in range(2):
                qs = (2 * mh, 2 * mh + 1)
                l2_ps = [ps_pool.tile([128, 2 * MQ], dt.float32,
                                      tag=f"hl2_{ch}", name=f"l2_ps_{mh}_{ch}")
                         for ch in range(2)]
                with tc.high_priority(offset=-1_000_000):
                    for ti, t in enumerate(t2):
                        for ch in range(2):
                            for q in qs:
                                nc.tensor.matmul(
                                    l2_ps[ch][:, (q % 2) * MQ:
                                              (q % 2 + 1) * MQ],
                                    z2_tile(t, ch),
                                    at_tile(t, q),
                                    start=(ti == 0 and q == qs[0]),
                                    stop=(ti == NT - 1),
                                )
                tail(1, list(qs),
                     lambda ch, q: l2_ps[ch][:, (q % 2) * MQ:(q % 2 + 1) * MQ],
                     l2_slice_done, contextlib.nullcontext())

    nc.compile()
    return nc


def _prep_inputs(x, net_params, A):
    a_bf = A.astype(BF16)
    z0 = np.ascontiguousarray(x.transpose(1, 0, 2).reshape(N, C)).astype(BF16)
    # z0 in SBUF tile order [k, p, t, c] -> [ZCH*128, TPZ*C]
    z0_sw = np.ascontiguousarray(
        z0.reshape(ZCH, TPZ, 128, C).transpose(0, 2, 1, 3)
    ).reshape(ZCH * 128, TPZ * C)
    w = net_params.astype(np.float32).reshape(L, F, F).astype(BF16)
    # block-diagonal weight tile per layer: diag(W_l, W_l)
    w_sb = np.zeros((128, 2 * 128), dtype=BF16)
    for li in range(L):
        w_sb[0:F, li * 128:li * 128 + F] = w[li]
        w_sb[F:2 * F, li * 128 + F:li * 128 + 2 * F] = w[li]
    in_maps = []
    for j in range(NCORES):
        at_j = np.ascontiguousarray(a_bf[j * M_CORE:(j + 1) * M_CORE, :].T)
        # A^T in SBUF tile order [q, k, p, t, m] -> [NQ*KCH*128, TPC*MQ]
        at_sw = np.ascontiguousarray(
            at_j.reshape(KCH, TPC, 128, NQ, MQ).transpose(3, 0, 2, 1, 4)
        ).reshape(NQ * KCH * 128, TPC * MQ)
        in_maps.append({"at": at_sw, "z0": z0_sw, "w": w_sb})
    return in_maps


def kernel(x, t, net_params, A):
    x = np.asarray(x)
    A = np.asarray(A)
    net_params = np.asarray(net_params)

    if "nc" not in _CACHED:
        _CACHED["nc"] = _build_program()
    nc = _CACHED["nc"]

    in_maps = _prep_inputs(x, net_params, A)
    _CACHED["in_maps"] = in_maps
    res = run_bass_kernel_spmd(nc, in_maps, list(range(NCORES)))
    full = np.concatenate([res.results[c]["out"] for c in range(NCORES)],
                          axis=0).astype(np.float32)
    return np.ascontiguousarray(full.reshape(N, B, F).transpose(1, 0, 2))


# revision 12
# speedup vs baseline: 1.1100x; 1.1100x over previous
"""Trainium2 Bass kernel for a 2-layer dense GCN block:

    z = x.reshape(B, N, F)                     # B=4, N=8192, F=64
    for i in range(2):
        z = relu((A @ z) @ W_i)                # A: [N, N] dense
    return z

Strategy (8 NeuronCores, SPMD):
  * Shard the output rows (m) of A @ Z across cores: core j owns rows
    [1024*j, 1024*(j+1)).  The host hands core j the matching
    column-slice of A^T (contraction dim n on SBUF partitions), cast to
    bf16 and pre-swizzled into exact SBUF tile order so every chunk DMA
    is one flat contiguous copy (short strided runs halve HBM rate).
    The 16 MB shard stays resident in SBUF for BOTH layers -- A is read
    from HBM exactly once.
  * Ring assignment is built around the inter-layer AllGather chain,
    whose ~30-40 us ncfw latency per collective is the critical path:
    z0 + z1_loc stores + outputs ride the sync HWDGE ring (so a
    gather's input store -- which gates its trigger doorbell -- never
    queues behind the big A load), while A + gather reloads ride the
    scalar ring (a reload waiting on its collective has nothing queued
    behind it there).
  * Z is a [n, c] matrix with c = b*F + f (256 columns).  Layer matmuls
    compute H^T[c, m] = sum_n Z[n, c] * A^T[n, m] on the tensor engine
    (lhsT = Z tile stationary, rhs = A^T tile moving, fp32 PSUM accum).
  * Layer 1 runs as FOUR m-quarter passes; quarter q's AllGather
    triggers ~1/4 of the way through the layer and the 4-slice chain
    pipelines under the rest of layer 1 and the start of layer 2.
    Both c-halves of a pass accumulate into ONE psum bank (per-element
    has_written handles the shared bank with a single start=True), so
    layers use disjoint psum tags and a later pass's tail can never be
    scheduler-ordered behind the next layer's matmuls.
  * Layer 2 accumulation matmuls are emitted at LOW scheduler priority
    so layer-1 tail work (weight-apply -> relu -> store -> gather
    trigger) always precedes them in the PE queue -- a reload-waiting
    LDWEIGHTS ahead of them would wedge the whole PE FIFO.  Layer 2
    consumes gather slices in arrival order, in two m-half passes.
  * bf16 operands / fp32 accumulation throughout (~0.5% rel-l2 vs the
    fp32 reference).
"""

import contextlib

import numpy as np
import ml_dtypes

import concourse.mybir as mybir
import concourse.tile as tile
from concourse import bacc
from concourse.bass_utils import run_bass_kernel_spmd

BF16 = ml_dtypes.bfloat16

NCORES = 8
B, N, F, L = 4, 8192, 64, 2
C = B * F                      # 256 columns of the Z matrix
M_CORE = N // NCORES           # 1024 output rows per core
NT = N // 128                  # 64 contraction tiles of 128
MT = M_CORE // 128             # 8 output-row tiles of 128 per core
KCH = 4                        # DMA chunks for the resident A^T shard
TPC = NT // KCH                # 16 n-tiles per A chunk
ZCH = 16                       # DMA chunks for z0 (small, for fast start)
TPZ = NT // ZCH                # 4 n-tiles per z chunk
NQ = 4                         # m-quarter passes / gather slices
MPG = MT // NQ                 # m-tiles per gather slice (2)
MQ = M_CORE // NQ              # m columns per quarter (256)

_CACHED = {}


def _build_program():
    nc = bacc.Bacc("TRN2", target_bir_lowering=False, debug=False,
                   num_devices=NCORES)
    dt = mybir.dt

    at_d = nc.dram_tensor("at", [NQ * KCH * 128, TPC * MQ], dt.bfloat16,
                          kind="ExternalInput")
    z0_d = nc.dram_tensor("z0", [ZCH * 128, TPZ * C], dt.bfloat16,
                          kind="ExternalInput")
    w_d = nc.dram_tensor("w", [128, 2 * 128], dt.bfloat16, kind="ExternalInput")
    out_d = nc.dram_tensor("out", [M_CORE, C], dt.bfloat16, kind="ExternalOutput")

    z1_loc = nc.dram_tensor("z1_loc", [M_CORE, C], dt.bfloat16)
    warm_in = nc.dram_tensor("warm_in", [1, 128], dt.bfloat16)
    warm_out = nc.dram_tensor("warm_out", [NCORES, 128], dt.bfloat16)
    z1g = [nc.dram_tensor(f"z1g{g}", [NCORES * MPG * 128, C], dt.bfloat16)
           for g in range(NQ)]

    with tile.TileContext(nc) as tc:
        with tc.tile_pool(name="a_res", bufs=1) as a_pool, \
             tc.tile_pool(name="z_res", bufs=1) as z_pool, \
             tc.tile_pool(name="z1_res", bufs=1) as z1_pool, \
             tc.tile_pool(name="wk", bufs=1) as w_pool, \
             tc.tile_pool(name="ps", bufs=1, space="PSUM") as ps_pool, \
             tc.tile_pool(name="pz", bufs=2, space="PSUM") as psz_pool, \
             tc.tile_pool(name="hsb", bufs=2) as hsb_pool, \
             tc.tile_pool(name="zout", bufs=4) as zout_pool:

            w_sb = w_pool.tile([128, 2 * 128], dt.bfloat16, tag="w")
            nc.scalar.dma_start(out=w_sb[:], in_=w_d[:])

            at_sb = [[a_pool.tile([128, TPC * MQ], dt.bfloat16,
                                  tag=f"at{q}_{k}", name=f"at_sb{q}_{k}")
                      for k in range(KCH)] for q in range(NQ)]
            z_sb = [z_pool.tile([128, TPZ * C], dt.bfloat16,
                                tag=f"z{k}", name=f"z_sb{k}")
                    for k in range(ZCH)]
            z1_sb = [z1_pool.tile([128, NCORES * MPG * C], dt.bfloat16,
                                  tag=f"z1g{g}", name=f"z1_sb{g}")
                     for g in range(NQ)]

            for k in range(ZCH):
                nc.sync.dma_start(out=z_sb[k][:],
                                  in_=z0_d[k * 128:(k + 1) * 128, :])
            for q in range(NQ):
                for k in range(KCH):
                    r = (q * KCH + k) * 128
                    nc.scalar.dma_start(out=at_sb[q][k][:],
                                        in_=at_d[r:r + 128, :])

            def z_tile(t, ch):
                """lhsT: Z[n-tile t, c-half ch] -> [128, 128] bf16."""
                k, tt = divmod(t, TPZ)
                return z_sb[k][:, tt * C + ch * 128: tt * C + ch * 128 + 128]

            def z2_tile(t, ch):
                """Same, from the gathered z1 slices."""
                cb, r = divmod(t, MT)
                g, tt = divmod(r, MPG)
                blk = cb * MPG + tt
                return z1_sb[g][:, blk * C + ch * 128: blk * C + ch * 128 + 128]

            def at_tile(t, q):
                """rhs: A^T[n-tile t, m-quarter q] -> [128, 256] bf16."""
                k, tt = divmod(t, TPC)
                return at_sb[q][k][:, tt * MQ:(tt + 1) * MQ]

            h_sb = [hsb_pool.tile([128, M_CORE], dt.bfloat16,
                                  tag=f"h{ch}", name=f"h_sb{ch}")
                    for ch in range(2)]

            def tail(li, qs, h_ap, on_slice_done, prio):
                # weight apply + relu + store for the gather slices of a
                # just-finished pass; overlaps the next pass's matmuls.
                with prio:
                    for ch in range(2):
                        for q in qs:
                            nc.vector.tensor_copy(
                                h_sb[ch][:, q * MQ:(q + 1) * MQ],
                                h_ap(ch, q),
                            )
                    for g in qs:
                        z_ps = psz_pool.tile([128, MPG * C], dt.float32,
                                             tag="zps", name=f"z_ps_{li}_{g}")
                        for j in range(MPG):
                            i = g * MPG + j
                            for ch in range(2):
                                nc.tensor.matmul(
                                    z_ps[:, j * C + ch * 128:
                                         j * C + (ch + 1) * 128],
                                    h_sb[ch][:, i * 128:(i + 1) * 128],
                                    w_sb[:, li * 128:(li + 1) * 128],
                                    start=(j == 0 and ch == 0), stop=True,
                                )
                        z_o = zout_pool.tile([128, MPG * C], dt.bfloat16,
                                             tag="zo", name=f"z_o_{li}_{g}")
                        nc.scalar.activation(z_o[:], z_ps[:],
                                             mybir.ActivationFunctionType.Relu)
                        on_slice_done(g, z_o)

            # Warm the ncfw collective path early (hidden under the A
            # load).  Keep it TINY: a full-size warmup crawls against
            # the loaded HBM and the real gathers queue behind it.
            nc.gpsimd.dma_start(out=warm_in[:], in_=z0_d[0:1, 0:128])
            nc.gpsimd.collective_compute(
                "AllGather",
                mybir.AluOpType.bypass,
                replica_groups=[list(range(NCORES))],
                ins=[warm_in.ap().opt()],
                outs=[warm_out.ap().opt()],
            )

            # ---- layer 1: four m-quarter passes ----
            def l1_slice_done(g, z_o):
                nc.sync.dma_start(
                    out=z1_loc.ap()[g * MPG * 128:(g + 1) * MPG * 128, :]
                        .rearrange("(t p) c -> p t c", p=128),
                    in_=z_o.rearrange("p (t c) -> p t c", c=C))
                nc.gpsimd.collective_compute(
                    "AllGather",
                    mybir.AluOpType.bypass,
                    replica_groups=[list(range(NCORES))],
                    ins=[z1_loc.ap()[g * MPG * 128:(g + 1) * MPG * 128, :].opt()],
                    outs=[z1g[g].ap().opt()],
                )
                # The reload must sort BEHIND every A chunk on its ring:
                # at high priority the scheduler parks it (waiting on its
                # collective) ahead of later A chunks in the ring FIFO,
                # stalling layer 1's last passes for tens of us.
                with tc.high_priority(offset=-1_000_000):
                    nc.scalar.dma_start(
                        out=z1_sb[g].rearrange("p (cb t c) -> p cb t c",
                                               cb=NCORES, t=MPG),
                        in_=z1g[g].ap().rearrange("(cb t p) c -> p cb t c",
                                                  cb=NCORES, p=128))

            l1_ps = [ps_pool.tile([128, 2 * MQ], dt.float32, tag=f"hl1_{par}",
                                  name=f"l1_ps{par}") for par in range(2)]
            for q in range(NQ):
                par = q % 2
                for ti, t in enumerate(range(NT)):
                    for ch in range(2):
                        nc.tensor.matmul(
                            l1_ps[par][:, ch * MQ:(ch + 1) * MQ],
                            z_tile(t, ch),
                            at_tile(t, q),
                            start=(ti == 0 and ch == 0),
                            stop=(ti == NT - 1),
                        )
                tail(0, [q],
                     lambda ch, _q, par=par: l1_ps[par][:, ch * MQ:(ch + 1) * MQ],
                     l1_slice_done, tc.high_priority())

            # ---- layer 2: two m-half passes, n-tiles in gather order ----
            t2 = [MT * cb + MPG * g + tt
                  for g in range(NQ) for cb in range(NCORES)
                  for tt in range(MPG)]

            def l2_slice_done(g, z_o):
                nc.sync.dma_start(
                    out=out_d.ap()[g * MPG * 128:(g + 1) * MPG * 128, :]
                        .rearrange("(t p) c -> p t c", p=128),
                    in_=z_o.rearrange("p (t c) -> p t c", c=C))

            for mh in range(2):
                qs = (2 * mh, 2 * mh + 1)
                l2_ps = [ps_pool.tile([128, 2 * MQ], dt.float32,
                                      tag=f"hl2_{ch}", name=f"l2_ps_{mh}_{ch}")
                         for ch in range(2)]
                with tc.high_priority(offset=-1_000_000):
                    for ti, t in enumerate(t2):
                        for ch in range(2):
                            for q in qs:
                                nc.tensor.matmul(
                                    l2_ps[ch][:, (q % 2) * MQ:
                                              (q % 2 + 1) * MQ],
                                    z2_tile(t, ch),
                                    at_tile(t, q),
                                    start=(ti == 0 and q == qs[0]),
                                    stop=(ti == NT - 1),
                                )
                tail(1, list(qs),
                     lambda ch, q: l2_ps[ch][:, (q % 2) * MQ:(q % 2 + 1) * MQ],
                     l2_slice_done, contextlib.nullcontext())

    nc.compile()
    return nc


def _prep_inputs(x, net_params, A):
    a_bf = A.astype(BF16)
    z0 = np.ascontiguousarray(x.transpose(1, 0, 2).reshape(N, C)).astype(BF16)
    # z0 in SBUF tile order [k, p, t, c] -> [ZCH*128, TPZ*C]
    z0_sw = np.ascontiguousarray(
        z0.reshape(ZCH, TPZ, 128, C).transpose(0, 2, 1, 3)
    ).reshape(ZCH * 128, TPZ * C)
    w = net_params.astype(np.float32).reshape(L, F, F).astype(BF16)
    # block-diagonal weight tile per layer: diag(W_l, W_l)
    w_sb = np.zeros((128, 2 * 128), dtype=BF16)
    for li in range(L):
        w_sb[0:F, li * 128:li * 128 + F] = w[li]
        w_sb[F:2 * F, li * 128 + F:li * 128 + 2 * F] = w[li]
    in_maps = []
    for j in range(NCORES):
        at_j = np.ascontiguousarray(a_bf[j * M_CORE:(j + 1) * M_CORE, :].T)
        # A^T in SBUF tile order [q, k, p, t, m] -> [NQ*KCH*128, TPC*MQ]
        at_sw = np.ascontiguousarray(
            at_j.reshape(KCH, TPC, 128, NQ, MQ).transpose(3, 0, 2, 1, 4)
        ).reshape(NQ * KCH * 128, TPC * MQ)
        in_maps.append({"at": at_sw, "z0": z0_sw, "w": w_sb})
    return in_maps


def kernel(x, t, net_params, A):
    x = np.asarray(x)
    A = np.asarray(A)
    net_params = np.asarray(net_params)

    if "nc" not in _CACHED:
        _CACHED["nc"] = _build_program()
    nc = _CACHED["nc"]

    in_maps = _prep_inputs(x, net_params, A)
    _CACHED["in_maps"] = in_maps
    res = run_bass_kernel_spmd(nc, in_maps, list(range(NCORES)))
    full = np.concatenate([res.results[c]["out"] for c in range(NCORES)],
                          axis=0).astype(np.float32)
    return np.ascontiguousarray(full.reshape(N, B, F).transpose(1, 0, 2))
